# revision 1
# baseline (speedup 1.0000x reference)
"""LBLHighwayBiLm Trainium2 kernel (8-core data-parallel over batch).

Layout: activations live as [D -> 2 blocks of 128 partitions, tokens on free
dim], bf16. Highway matmuls run on PE (W^T stationary, bf16, N=1024 moving
chunks into PSUM); sigmoid (+bias) and relu (+bias) evict PSUM on ACT; the
highway combine and the 5-tap causal/anticausal convs run on DVE/GPSIMD with
fp32 tap weights as immediates. Conv boundary handling via per-row halo
columns (host-prepadded for layer 0, constant pad tiles for layer 1).
Each core handles 4 batch rows; no collectives.
"""

import numpy as np
import ml_dtypes

import concourse.bacc as bacc
import concourse.tile as tile
import concourse.mybir as mybir
from concourse.bass_utils import run_bass_kernel_spmd

BF16 = mybir.dt.bfloat16
F32 = mybir.dt.float32
AOP = mybir.AluOpType
AFT = mybir.ActivationFunctionType

N_LAYERS = 2
N_HW = 2
W = 4
D = 256
B, S = 32, 1024
NCORES = 8
BLOC = B // NCORES          # 4 batch rows per core
T = BLOC * S                # 4096 tokens per core
PB = D // 128               # 2 partition blocks for D
EB = (2 * D) // 128         # 4 partition blocks for 2D
ROW0 = S + 2 * W            # layer-0 padded row (front+back halo): 1032
ROW1 = S + W                # layer-1 padded row (one halo): 1028
CH = 1024                   # token chunk = one batch row

# --- engine assignment knobs -------------------------------------------------
# conv taps 1..4 (tap 0 is tensor_scalar on DVE): engine per tap
CONV_TAP_ENGINE = ["v", "v", "v", "v"]   # STT is DVE-only (Pool lacks the opcode)
ENG_T = "v"    # t = f - r
ENG_U = "g"    # u = g * t
ENG_X1 = "v"   # x1 = u + r
RELU_ENGINE = "a"  # "a"=ACT (reads PSUM + free bias)
MM_N = 1024    # moving free dim per matmul (bf16 allows 1024)
SCRATCH_BUFS = 9
SKIP_CONV = False
SKIP_MM = False


def _eng(nc, code):
    return {"v": nc.vector, "g": nc.gpsimd}[code]


def build_bass(params):
    """params: dict of host-precomputed constant arrays / floats."""
    nc = bacc.Bacc(target_bir_lowering=False)

    x_in = nc.dram_tensor("x", [PB, 128, BLOC * ROW0], F32, kind="ExternalInput")
    out = nc.dram_tensor(
        "out", [N_LAYERS, 2, PB, 128, T], BF16, kind="ExternalOutput"
    )

    # inline constants
    wt_dram = nc.inline_tensor(params["wt"], name="wt")        # [L,2,HW,PB,128,512] bf16
    bias_dram = nc.inline_tensor(params["bias"], name="bias")  # [128, L*2*HW*EB] f32
    pad_dram = nc.inline_tensor(params["pad1"], name="pad1")   # [128, 2*PB*W] bf16
    fw = params["fwd_w"]  # [L, W+1] python floats
    bw = params["bwd_w"]

    with tile.TileContext(nc) as tc:
        consts = tc.alloc_tile_pool(name="consts", bufs=1)
        bufs = tc.alloc_tile_pool(name="bufs", bufs=1)
        scratch = tc.alloc_tile_pool(name="scratch", bufs=SCRATCH_BUFS)
        psum = tc.alloc_tile_pool(name="psum", bufs=4, space="PSUM")

        # ---- load constants -------------------------------------------------
        wt_sb = {}
        for l in range(N_LAYERS):
            for di in range(2):
                for h in range(N_HW):
                    for kb in range(PB):
                        wtt = consts.tile(
                            [128, 2 * D], BF16, tag=f"wt{l}{di}{h}{kb}",
                            name=f"wt{l}{di}{h}{kb}",
                        )
                        nc.sync.dma_start(out=wtt, in_=wt_dram[l, di, h, kb])
                        wt_sb[(l, di, h, kb)] = wtt
        bias_sb = consts.tile([128, N_LAYERS * 2 * N_HW * EB], F32, name="bias_sb")
        nc.sync.dma_start(out=bias_sb, in_=bias_dram[:, :])
        pad_sb = consts.tile([128, 2 * PB * W], BF16, name="pad_sb")
        nc.sync.dma_start(out=pad_sb, in_=pad_dram[:, :])

        def bias_ap(l, di, h, eb):
            i = ((l * 2 + di) * N_HW + h) * EB + eb
            return bias_sb[:, i:i + 1]

        # ---- layer-0 padded input (cast-load fp32 -> bf16) ------------------
        xpad0 = []
        for blk in range(PB):
            xp = bufs.tile([128, BLOC * ROW0], BF16, tag=f"xpad0_{blk}",
                           name=f"xpad0_{blk}")
            xpad0.append(xp)
        for blk in range(PB):
            for r in range(BLOC):
                nc.gpsimd.dma_start(
                    out=xpad0[blk][:, r * ROW0:(r + 1) * ROW0],
                    in_=x_in[blk, :, r * ROW0:(r + 1) * ROW0],
                )

        # conv sources per layer: (tiles, row_len, fwd_data_off, bwd_data_off)
        # layer0 rows: [front(4) | x(1024) | back(4)], fwd taps at col j,
        # bwd taps at col 4+j.
        def conv(dst, src, row_len, base_off, taps):
            """dst[blk][:, r*S:(r+1)*S] = sum_j taps[j]*src[blk][:, r*row_len+base_off+j : +S]"""
            for blk in range(PB):
                for r in range(BLOC):
                    acc = dst[blk][:, r * CH:(r + 1) * CH]
                    def sl(j):
                        o = r * row_len + base_off + j
                        return src[blk][:, o:o + CH]
                    nc.vector.tensor_scalar_mul(acc, sl(0), float(taps[0]))
                    if SKIP_CONV:
                        continue
                    for j in range(1, W + 1):
                        eng = _eng(nc, CONV_TAP_ENGINE[j - 1])
                        eng.scalar_tensor_tensor(
                            acc, sl(j), float(taps[j]), acc, AOP.mult, AOP.add
                        )

        # ---- one highway sublayer ------------------------------------------
        def highway(l, di, h, x0, x1, x1_row_len, x1_off):
            """x1[blk] rows written from x0 [PB][128, T]; x1 may be padded
            (row_len/off) or plain (row_len=CH, off=0 with contiguous rows)."""
            for tg in range(T // MM_N):
                ps = {}
                for eb in range(EB):
                    p = psum.tile([128, MM_N], F32, tag="ps", name=f"ps{l}{di}{h}{eb}{tg}")
                    for half in range(MM_N // 512):
                        for kb in range(PB if not SKIP_MM else 1):
                            nc.tensor.matmul(
                                p[:, half * 512:(half + 1) * 512],
                                lhsT=wt_sb[(l, di, h, kb)][:, eb * 128:(eb + 1) * 128],
                                rhs=x0[kb][:, tg * MM_N + half * 512:tg * MM_N + (half + 1) * 512],
                                start=(kb == 0),
                                stop=(kb == PB - 1),
                            )
                    ps[eb] = p
                # nonlin = eblks [0, PB), gate = eblks [PB, 2*PB)
                for blk in range(PB):
                    gt = scratch.tile([128, MM_N], BF16, tag="g", name=f"g{l}{di}{h}{blk}{tg}")
                    nc.scalar.activation(
                        gt, ps[PB + blk], AFT.Sigmoid,
                        bias=bias_ap(l, di, h, PB + blk), scale=1.0,
                    )
                    rt = scratch.tile([128, MM_N], BF16, tag="r", name=f"r{l}{di}{h}{blk}{tg}")
                    if RELU_ENGINE == "a":
                        nc.scalar.activation(
                            rt, ps[blk], AFT.Relu,
                            bias=bias_ap(l, di, h, blk), scale=1.0,
                        )
                    else:
                        # (nl + bias) max 0 — one fused tensor_scalar
                        _eng(nc, RELU_ENGINE).tensor_scalar(
                            rt, ps[blk], bias_ap(l, di, h, blk), 0.0,
                            AOP.add, AOP.max,
                        )
                    tt = scratch.tile([128, MM_N], BF16, tag="t", name=f"t{l}{di}{h}{blk}{tg}")
                    x0c = x0[blk][:, tg * MM_N:(tg + 1) * MM_N]
                    _eng(nc, ENG_T).tensor_tensor(tt, x0c, rt, AOP.subtract)
                    ut = scratch.tile([128, MM_N], BF16, tag="u", name=f"u{l}{di}{h}{blk}{tg}")
                    _eng(nc, ENG_U).tensor_tensor(ut, gt, tt, AOP.mult)
                    # x1 destination chunk (MM_N == CH == one batch row)
                    o = tg * x1_row_len + x1_off
                    x1c = x1[blk][:, o:o + MM_N]
                    _eng(nc, ENG_X1).tensor_tensor(x1c, ut, rt, AOP.add)

        # ---- the network ----------------------------------------------------
        f_t = {}   # conv outputs per dir
        x1a = {}   # sublayer-A outputs per dir
        for l in range(N_LAYERS):
            # conv inputs for this layer
            if l == 0:
                src = {0: (xpad0, ROW0, 0), 1: (xpad0, ROW0, W)}
            else:
                src = {0: (xpadf, ROW1, 0), 1: (xpadb, ROW1, 0)}
            for di in range(2):
                taps = fw[l] if di == 0 else bw[l]
                ft = [
                    bufs.tile([128, T], BF16, tag=f"f{di}{blk}", name=f"f{l}{di}{blk}")
                    for blk in range(PB)
                ]
                s_tiles, rl, off = src[di]
                conv(ft, s_tiles, rl, off, taps)
                f_t[di] = ft

            # allocate next-layer padded buffers (written by sublayer B)
            if l == 0:
                xpadf = [
                    bufs.tile([128, BLOC * ROW1], BF16, tag=f"xpf{blk}", name=f"xpf{blk}")
                    for blk in range(PB)
                ]
                xpadb = [
                    bufs.tile([128, BLOC * ROW1], BF16, tag=f"xpb{blk}", name=f"xpb{blk}")
                    for blk in range(PB)
                ]
                # halos: fwd front cols [0,W), bwd back cols [S, S+W)
                for blk in range(PB):
                    for r in range(BLOC):
                        nc.vector.tensor_copy(
                            xpadf[blk][:, r * ROW1:r * ROW1 + W],
                            pad_sb[:, (0 * PB + blk) * W:(0 * PB + blk + 1) * W],
                        )
                        nc.vector.tensor_copy(
                            xpadb[blk][:, r * ROW1 + S:(r + 1) * ROW1],
                            pad_sb[:, (1 * PB + blk) * W:(1 * PB + blk + 1) * W],
                        )

            for di in range(2):
                xa = [
                    bufs.tile([128, T], BF16, tag=f"xa{di}{blk}", name=f"xa{l}{di}{blk}")
                    for blk in range(PB)
                ]
                highway(l, di, 0, f_t[di], xa, CH, 0)
                x1a[di] = xa

            for di in range(2):
                if l == 0:
                    x1 = xpadf if di == 0 else xpadb
                    rl, off = ROW1, (W if di == 0 else 0)
                else:
                    # reuse the (now dead) conv-output slots for the final out
                    x1 = [
                        bufs.tile([128, T], BF16, tag=f"f{di}{blk}", name=f"xb{l}{di}{blk}")
                        for blk in range(PB)
                    ]
                    rl, off = CH, 0
                highway(l, di, 1, x1a[di], x1, rl, off)
                # DMA the layer output (strided rows for l==0 padded bufs)
                for blk in range(PB):
                    src_ap = x1[blk].rearrange("p (r c) -> p r c", c=rl)[:, :, off:off + CH]
                    dst_ap = out[l, di, blk].rearrange("p (r c) -> p r c", c=CH)
                    nc.sync.dma_start(out=dst_ap, in_=src_ap)

        psum.release()
        scratch.release()
        bufs.release()
        consts.release()

    nc.finalize()
    return nc


def _prep_params(inputs):
    fwd_hw_W = np.asarray(inputs["fwd_hw_W"], np.float32)
    bwd_hw_W = np.asarray(inputs["bwd_hw_W"], np.float32)
    # lhsT layout: [l, dir, hw, kb, 128(k), 2D(e)] = W[e, k] transposed
    wt = np.empty((N_LAYERS, 2, N_HW, PB, 128, 2 * D), np.float32)
    for l in range(N_LAYERS):
        for di, Wsrc in ((0, fwd_hw_W), (1, bwd_hw_W)):
            for h in range(N_HW):
                wT = Wsrc[l, h].T  # [D, 2D]
                wt[l, di, h] = wT.reshape(PB, 128, 2 * D)
    wt = wt.astype(ml_dtypes.bfloat16)

    fwd_hw_b = np.asarray(inputs["fwd_hw_b"], np.float32)
    bwd_hw_b = np.asarray(inputs["bwd_hw_b"], np.float32)
    bias = np.empty((128, N_LAYERS * 2 * N_HW * EB), np.float32)
    for l in range(N_LAYERS):
        for di, bsrc in ((0, fwd_hw_b), (1, bwd_hw_b)):
            for h in range(N_HW):
                for eb in range(EB):
                    i = ((l * 2 + di) * N_HW + h) * EB + eb
                    bias[:, i] = bsrc[l, h, eb * 128:(eb + 1) * 128]

    # layer-1 halos: fwd front = fwd_pad[1].T, bwd back = bwd_pad[1].T
    fwd_pad = np.asarray(inputs["fwd_pad"], np.float32)
    bwd_pad = np.asarray(inputs["bwd_pad"], np.float32)
    pad1 = np.empty((128, 2 * PB * W), np.float32)
    for di, psrc in ((0, fwd_pad), (1, bwd_pad)):
        pT = psrc[1].T.reshape(PB, 128, W)  # [D, W] -> blocks
        for blk in range(PB):
            pad1[:, (di * PB + blk) * W:(di * PB + blk + 1) * W] = pT[blk]
    pad1 = pad1.astype(ml_dtypes.bfloat16)

    return {
        "wt": np.ascontiguousarray(wt),
        "bias": np.ascontiguousarray(bias),
        "pad1": np.ascontiguousarray(pad1),
        "fwd_w": [[float(v) for v in row] for row in np.asarray(inputs["fwd_w"], np.float32)],
        "bwd_w": [[float(v) for v in row] for row in np.asarray(inputs["bwd_w"], np.float32)],
    }


def _prep_core_input(x_core, fwd_pad, bwd_pad):
    """x_core: [BLOC, S, D] f32 -> [PB, 128, BLOC*ROW0] f32 with halos."""
    xt = np.ascontiguousarray(x_core.transpose(2, 0, 1))  # [D, BLOC, S]
    blocks = xt.reshape(PB, 128, BLOC, S)
    padded = np.empty((PB, 128, BLOC, ROW0), np.float32)
    padded[:, :, :, W:W + S] = blocks
    fr = fwd_pad[0].T.reshape(PB, 128, W)   # front halo (layer 0)
    bk = bwd_pad[0].T.reshape(PB, 128, W)
    padded[:, :, :, :W] = fr[:, :, None, :]
    padded[:, :, :, W + S:] = bk[:, :, None, :]
    return np.ascontiguousarray(padded.reshape(PB, 128, BLOC * ROW0))


_NC_CACHE = {}


def kernel(**inputs):
    params = _prep_params(inputs)
    import hashlib
    h = hashlib.sha256()
    for k in ("wt", "bias", "pad1"):
        h.update(params[k].tobytes())
    h.update(repr(params["fwd_w"]).encode())
    h.update(repr(params["bwd_w"]).encode())
    key = h.hexdigest()
    if key not in _NC_CACHE:
        _NC_CACHE[key] = build_bass(params)
    nc = _NC_CACHE[key]

    x = np.asarray(inputs["inputs"], np.float32)
    fwd_pad = np.asarray(inputs["fwd_pad"], np.float32)
    bwd_pad = np.asarray(inputs["bwd_pad"], np.float32)
    in_maps = [
        {"x": _prep_core_input(x[c * BLOC:(c + 1) * BLOC], fwd_pad, bwd_pad)}
        for c in range(NCORES)
    ]
    res = run_bass_kernel_spmd(nc, in_maps, core_ids=list(range(NCORES)))

    y = np.empty((N_LAYERS, B, S, 2 * D), np.float32)
    for c in range(NCORES):
        o = np.asarray(res.results[c]["out"]).astype(np.float32)
        # [L, dir, blk, p, T] -> [L, r, s, dir*256+blk*128+p]
        o = o.reshape(N_LAYERS, 2, PB, 128, BLOC, S)
        o = o.transpose(0, 4, 5, 1, 2, 3).reshape(N_LAYERS, BLOC, S, 2 * D)
        y[:, c * BLOC:(c + 1) * BLOC] = o
    return y



# revision 14
# speedup vs baseline: 1.0640x; 1.0640x over previous
"""LBLHighwayBiLm Trainium2 kernel (8-core data-parallel over batch).

v2: the 5-tap causal/anticausal convs run on the PE as banded token-mixing
matmuls. Activations for the conv live in token-partition layout ("x_T":
128 tokens on partitions, blocks x 256 chans on free dim); each 128-token
block is one [K=128] band matmul plus a [K=4] fixup matmul whose rhs is
either the neighbouring block's edge tokens or the (constant) pad vectors.
Conv PSUM is evicted to bf16 SBUF (DVE) and DMA-transposed back to
channel-partition layout for the highway matmuls.

Highway sublayers: W^T stationary bf16 matmuls into PSUM; sigmoid/relu
(+bias) evict PSUM on ACT; combine t=x-r, x1=u+r on DVE, u=g*t on GPSIMD
via the STT form (cheaper than TT there). fwd/bwd instruction emission is
interleaved to keep every engine (esp. PE pstate) busy.

Each core handles 4 batch rows; no collectives.
"""

import numpy as np
import ml_dtypes

import concourse.bacc as bacc
import concourse.tile as tile
import concourse.mybir as mybir
from concourse.bass_utils import run_bass_kernel_spmd

BF16 = mybir.dt.bfloat16
F32 = mybir.dt.float32
AOP = mybir.AluOpType
AFT = mybir.ActivationFunctionType

N_LAYERS = 2
N_HW = 2
W = 4
D = 256
B, S = 32, 1024
NCORES = 8
BLOC = B // NCORES          # 4 batch rows per core
T = BLOC * S                # 4096 tokens per core
PB = D // 128               # 2 partition blocks for D
EB = (2 * D) // 128         # 4 partition blocks for 2D
CH = 1024                   # token chunk = one batch row
NBLK = T // 128             # 32 token blocks of 128
BPR = S // 128              # 8 blocks per batch row

# --- engine assignment knobs -------------------------------------------------
ENG_T = ["v", "v", "v", "v"]       # t = x0 - r, per tg
ENG_U = ["g", "g", "g", "g"]       # u = g * t  ("g"=pool STT form)
ENG_X1 = ["v", "v", "g", "v"]      # x1 = u + r
RELU_ENGINE = ["a", "a", "a", "v"]  # per tg: "a"=ACT, "v"=DVE tensor_scalar
EVICT_ENGINE = "v"                  # conv psum evict


def _tt(nc, code, out, a, b, op):
    """tensor_tensor on DVE ("v") or gpsimd ("g")."""
    eng = nc.vector if code == "v" else nc.gpsimd
    eng.tensor_tensor(out, a, b, op)


def build_bass(params):
    """params: dict of host-precomputed constant arrays / floats."""
    nc = bacc.Bacc(target_bir_lowering=False)

    x_in = nc.dram_tensor("x", [128, NBLK * 256], BF16, kind="ExternalInput")
    out = nc.dram_tensor(
        "out", [N_LAYERS, 2, PB, 128, T], BF16, kind="ExternalOutput"
    )

    # inline constants
    wt_dram = nc.inline_tensor(params["wt"], name="wt")        # [L,2,HW,PB,128,512] bf16
    bias_dram = nc.inline_tensor(params["bias"], name="bias")  # [128, L*2*HW*EB] f32
    band_dram = nc.inline_tensor(params["band"], name="band")  # [L,2,2,128,128] bf16
    pad_dram = nc.inline_tensor(params["pad"], name="pad")     # [L,2,4,256] bf16

    with tile.TileContext(nc) as tc:
        consts = tc.alloc_tile_pool(name="consts", bufs=1)
        bufs = tc.alloc_tile_pool(name="bufs", bufs=1)
        scratch = tc.alloc_tile_pool(name="scratch", bufs=5)
        psum = tc.alloc_tile_pool(name="psum", bufs=4, space="PSUM")

        # ---- load constants -------------------------------------------------
        wt_sb = {}
        for l in range(N_LAYERS):
            for di in range(2):
                for h in range(N_HW):
                    for kb in range(PB):
                        wtt = consts.tile(
                            [128, 2 * D], BF16, tag=f"wt{l}{di}{h}{kb}",
                            name=f"wt{l}{di}{h}{kb}",
                        )
                        nc.sync.dma_start(out=wtt, in_=wt_dram[l, di, h, kb])
                        wt_sb[(l, di, h, kb)] = wtt
        bias_sb = consts.tile([128, N_LAYERS * 2 * N_HW * EB], F32, name="bias_sb")
        nc.sync.dma_start(out=bias_sb, in_=bias_dram[:, :])
        band_sb = {}   # (l, di, fix) -> [128,128] tile (fix uses partitions 0..3)
        for l in range(N_LAYERS):
            for di in range(2):
                for fx in range(2):
                    bt = consts.tile([128, 128], BF16, tag=f"bd{l}{di}{fx}",
                                     name=f"bd{l}{di}{fx}")
                    nc.sync.dma_start(out=bt, in_=band_dram[l, di, fx])
                    band_sb[(l, di, fx)] = bt
        pad_sb = {}    # (l, di) -> [4?,256] pad tile on partitions 0..3
        for l in range(N_LAYERS):
            for di in range(2):
                pt = consts.tile([128, 256], BF16, tag=f"pd{l}{di}",
                                 name=f"pd{l}{di}")
                nc.sync.dma_start(out=pt[:4, :], in_=pad_dram[l, di])
                pad_sb[(l, di)] = pt

        def bias_ap(l, di, h, eb):
            i = ((l * 2 + di) * N_HW + h) * EB + eb
            return bias_sb[:, i:i + 1]

        # ---- layer-0 token-layout input ------------------------------------
        xT0 = bufs.tile([128, NBLK * 256], BF16, tag="xT0", name="xT0")
        for r in range(BLOC):
            nc.gpsimd.dma_start(
                out=xT0[:, r * BPR * 256:(r + 1) * BPR * 256],
                in_=x_in[:, r * BPR * 256:(r + 1) * BPR * 256],
            )
        # fwd-conv fixup edges: edg[k, b*256+c] = xT[124+k, (b-1)*256+c]
        edg0 = bufs.tile([4, NBLK * 256], BF16, tag="edg0", name="edg0")
        nc.sync.dma_start(
            out=edg0[0:4, 256:NBLK * 256],
            in_=xT0[124:128, 0:(NBLK - 1) * 256],
        )

        # ---- conv on PE -----------------------------------------------------
        def conv_group(l, di, xT, g, fT, edg):
            """One psum group: token blocks [4g, 4g+4) of the banded conv.
            Evicts into fT halves [128, NBLK*128] at [:, g*512:(g+1)*512]."""
            ps = psum.tile([128, 1024], F32, tag="ps", name=f"cps{l}{di}{g}")
            main = band_sb[(l, di, 0)]
            fix = band_sb[(l, di, 1)]
            for i in range(4):
                b = 4 * g + i
                nc.tensor.matmul(
                    ps[:, i * 256:(i + 1) * 256],
                    lhsT=main, rhs=xT[:, b * 256:(b + 1) * 256],
                    start=True, stop=False,
                )
                if di == 0:
                    if b % BPR == 0:
                        rhs = pad_sb[(l, 0)][:4, :]
                    else:
                        rhs = edg[0:4, b * 256:(b + 1) * 256]
                else:
                    if b % BPR == BPR - 1:
                        rhs = pad_sb[(l, 1)][:4, :]
                    else:
                        rhs = xT[0:4, (b + 1) * 256:(b + 2) * 256]
                nc.tensor.matmul(
                    ps[:, i * 256:(i + 1) * 256],
                    lhsT=fix[:4, :], rhs=rhs,
                    start=False, stop=True,
                )
            # evict both chan-halves: fT_h[t, b*128+c] = ps[t, (b%4)*256+h*128+c]
            for h in range(2):
                src = ps.rearrange("p (i c) -> p i c", c=256)[:, :, h * 128:(h + 1) * 128]
                dst = fT[h][:, g * 512:(g + 1) * 512].rearrange(
                    "p (i c) -> p i c", c=128)
                if EVICT_ENGINE == "v":
                    nc.vector.tensor_copy(dst, src)
                else:
                    nc.scalar.copy(dst, src)

        # ---- one highway sublayer chunk ------------------------------------
        def hw_chunk(l, di, h, tg, x0, x1):
            """One 1024-token chunk of highway sublayer (l, di, h)."""
            pss = {}
            # nonlin ebs first, then gate ebs
            for eb in (0, 1, 2, 3):
                p = psum.tile([128, 1024], F32, tag="ps", name=f"hps{l}{di}{h}{tg}{eb}")
                for half in range(2):
                    for kb in range(PB):
                        nc.tensor.matmul(
                            p[:, half * 512:(half + 1) * 512],
                            lhsT=wt_sb[(l, di, h, kb)][:, eb * 128:(eb + 1) * 128],
                            rhs=x0[kb][:, tg * CH + half * 512:tg * CH + (half + 1) * 512],
                            start=(kb == 0),
                            stop=(kb == PB - 1),
                        )
                pss[eb] = p
            gt, rt = {}, {}
            for blk in range(PB):
                rt[blk] = scratch.tile([128, CH], BF16, tag="r", name=f"r{l}{di}{h}{blk}{tg}")
                if RELU_ENGINE[tg] == "a":
                    nc.scalar.activation(
                        rt[blk], pss[blk], AFT.Relu,
                        bias=bias_ap(l, di, h, blk), scale=1.0,
                    )
                else:
                    nc.vector.tensor_scalar(
                        rt[blk], pss[blk], bias_ap(l, di, h, blk), 0.0,
                        AOP.add, AOP.max,
                    )
            for blk in range(PB):
                gt[blk] = scratch.tile([128, CH], BF16, tag="g", name=f"g{l}{di}{h}{blk}{tg}")
                nc.scalar.activation(
                    gt[blk], pss[PB + blk], AFT.Sigmoid,
                    bias=bias_ap(l, di, h, PB + blk), scale=1.0,
                )
            for blk in range(PB):
                x0c = x0[blk][:, tg * CH:(tg + 1) * CH]
                tt = scratch.tile([128, CH], BF16, tag="t", name=f"t{l}{di}{h}{blk}{tg}")
                _tt(nc, ENG_T[tg], tt, x0c, rt[blk], AOP.subtract)
                ut = scratch.tile([128, CH], BF16, tag="u", name=f"u{l}{di}{h}{blk}{tg}")
                _tt(nc, ENG_U[tg], ut, gt[blk], tt, AOP.mult)
                x1c = x1[blk][:, tg * CH:(tg + 1) * CH]
                _tt(nc, ENG_X1[tg], x1c, ut, rt[blk], AOP.add)

        # ---- the network ----------------------------------------------------
        xT = {0: xT0, 1: xT0}
        edg = {0: edg0, 1: edg0}
        for l in range(N_LAYERS):
            # conv: banded matmuls, fwd/bwd interleaved
            fT = {}
            f = {}
            for di in range(2):
                fT[di] = [
                    bufs.tile([128, NBLK * 128], BF16, tag=f"fT{di}{h}",
                              name=f"fT{l}{di}{h}")
                    for h in range(2)
                ]
                f[di] = [
                    bufs.tile([128, T], BF16, tag=f"f{di}{blk}", name=f"f{l}{di}{blk}")
                    for blk in range(PB)
                ]
            for g in range(NBLK // 4):
                for di in range(2):
                    conv_group(l, di, xT[di], g, fT[di], edg[di])
                if g % 2 == 1:
                    tg = g // 2
                    for di in range(2):
                        for h in range(2):
                            nc.sync.dma_start_transpose(
                                out=f[di][h][:, tg * CH:(tg + 1) * CH]
                                    .rearrange("p (a b) -> p a b", b=128),
                                in_=fT[di][h][:, tg * CH:(tg + 1) * CH],
                            )

            # highway sublayer A
            # xa reuses the fT buffers (same shape, dead once back-transposed)
            xa = {}
            for di in range(2):
                xa[di] = [
                    bufs.tile([128, T], BF16, tag=f"fT{di}{blk}", name=f"xa{l}{di}{blk}")
                    for blk in range(PB)
                ]
            for tg in range(4):
                for di in range(2):
                    hw_chunk(l, di, 0, tg, f[di], xa[di])

            # highway sublayer B (+ output store, + next-layer transposes)
            xb = {}
            for di in range(2):
                xb[di] = [
                    bufs.tile([128, T], BF16, tag=f"xb{di}{blk}", name=f"xb{l}{di}{blk}")
                    for blk in range(PB)
                ]
                if l + 1 < N_LAYERS:
                    xT[di] = bufs.tile([128, NBLK * 256], BF16,
                                       tag="xT0" if di == 0 else "xT1b",
                                       name=f"xT1{di}")
                    if di == 0:
                        edg[di] = bufs.tile([4, NBLK * 256], BF16, tag="edg0",
                                            name="edg1")
            for tg in range(4):
                for di in range(2):
                    hw_chunk(l, di, 1, tg, xa[di], xb[di])
                    if l + 1 < N_LAYERS:
                        for h in range(2):
                            nc.sync.dma_start_transpose(
                                out=xT[di].rearrange("p (a b) -> p a b", b=256)
                                    [:, tg * BPR:(tg + 1) * BPR, h * 128:(h + 1) * 128],
                                in_=xb[di][h][:, tg * CH:(tg + 1) * CH],
                            )
                        if di == 0:
                            b0 = tg * BPR
                            nc.sync.dma_start(
                                out=edg[0][0:4, (b0 + 1) * 256:(b0 + BPR) * 256],
                                in_=xT[0][124:128, b0 * 256:(b0 + BPR - 1) * 256],
                            )
                    for blk in range(PB):
                        nc.sync.dma_start(
                            out=out[l, di, blk].rearrange("p (r c) -> p r c", c=CH)[:, tg],
                            in_=xb[di][blk][:, tg * CH:(tg + 1) * CH],
                        )

        psum.release()
        scratch.release()
        bufs.release()
        consts.release()

    nc.finalize()
    return nc


def _prep_params(inputs):
    fwd_hw_W = np.asarray(inputs["fwd_hw_W"], np.float32)
    bwd_hw_W = np.asarray(inputs["bwd_hw_W"], np.float32)
    # lhsT layout: [l, dir, hw, kb, 128(k), 2D(e)] = W[e, k] transposed
    wt = np.empty((N_LAYERS, 2, N_HW, PB, 128, 2 * D), np.float32)
    for l in range(N_LAYERS):
        for di, Wsrc in ((0, fwd_hw_W), (1, bwd_hw_W)):
            for h in range(N_HW):
                wT = Wsrc[l, h].T  # [D, 2D]
                wt[l, di, h] = wT.reshape(PB, 128, 2 * D)
    wt = wt.astype(ml_dtypes.bfloat16)

    fwd_hw_b = np.asarray(inputs["fwd_hw_b"], np.float32)
    bwd_hw_b = np.asarray(inputs["bwd_hw_b"], np.float32)
    bias = np.empty((128, N_LAYERS * 2 * N_HW * EB), np.float32)
    for l in range(N_LAYERS):
        for di, bsrc in ((0, fwd_hw_b), (1, bwd_hw_b)):
            for h in range(N_HW):
                for eb in range(EB):
                    i = ((l * 2 + di) * N_HW + h) * EB + eb
                    bias[:, i] = bsrc[l, h, eb * 128:(eb + 1) * 128]

    # banded conv weights: main [128,128] + fixup [4,128] (stored in 128 rows)
    fwd_w = np.asarray(inputs["fwd_w"], np.float32)
    bwd_w = np.asarray(inputs["bwd_w"], np.float32)
    band = np.zeros((N_LAYERS, 2, 2, 128, 128), np.float32)
    for l in range(N_LAYERS):
        for e in range(128):
            for p in range(128):
                d = p - e
                if -W <= d <= 0:
                    band[l, 0, 0, p, e] = fwd_w[l, d + W]   # w_{p-e+4}
                if 0 <= d <= W:
                    band[l, 1, 0, p, e] = bwd_w[l, d]       # v_{p-e}
        for e in range(128):
            for k in range(4):
                j = k - e
                if 0 <= j <= 3:
                    band[l, 0, 1, k, e] = fwd_w[l, j]       # w_{k-e}
                j = 128 + k - e
                if 1 <= j <= 4:
                    band[l, 1, 1, k, e] = bwd_w[l, j]       # v_{128+k-e}
    band = band.astype(ml_dtypes.bfloat16)

    fwd_pad = np.asarray(inputs["fwd_pad"], np.float32)
    bwd_pad = np.asarray(inputs["bwd_pad"], np.float32)
    pad = np.stack([fwd_pad, bwd_pad], axis=1)  # [L, 2, 4, 256]
    pad = np.ascontiguousarray(pad).astype(ml_dtypes.bfloat16)

    return {
        "wt": np.ascontiguousarray(wt),
        "bias": np.ascontiguousarray(bias),
        "band": np.ascontiguousarray(band),
        "pad": pad,
    }


def _prep_core_input(x_core):
    """x_core: [BLOC, S, D] f32 -> x_T [128, NBLK*256] bf16 token layout."""
    # x_T[t, b*256 + c] = x_core[b // BPR, (b % BPR)*128 + t, c]
    xt = x_core.reshape(BLOC * BPR, 128, D)          # [b, t, c]
    xt = np.ascontiguousarray(xt.transpose(1, 0, 2))  # [t, b, c]
    return xt.reshape(128, NBLK * D).astype(ml_dtypes.bfloat16)


_NC_CACHE = {}


def kernel(**inputs):
    params = _prep_params(inputs)
    import hashlib
    h = hashlib.sha256()
    for k in ("wt", "bias", "band", "pad"):
        h.update(np.ascontiguousarray(params[k]).tobytes())
    key = h.hexdigest()
    if key not in _NC_CACHE:
        _NC_CACHE[key] = build_bass(params)
    nc = _NC_CACHE[key]

    x = np.asarray(inputs["inputs"], np.float32)
    in_maps = [
        {"x": _prep_core_input(x[c * BLOC:(c + 1) * BLOC])}
        for c in range(NCORES)
    ]
    res = run_bass_kernel_spmd(nc, in_maps, core_ids=list(range(NCORES)))

    y = np.empty((N_LAYERS, B, S, 2 * D), np.float32)
    for c in range(NCORES):
        o = np.asarray(res.results[c]["out"]).astype(np.float32)
        # [L, dir, blk, p, T] -> [L, r, s, dir*256+blk*128+p]
        o = o.reshape(N_LAYERS, 2, PB, 128, BLOC, S)
        o = o.transpose(0, 4, 5, 1, 2, 3).reshape(N_LAYERS, BLOC, S, 2 * D)
        y[:, c * BLOC:(c + 1) * BLOC] = o
    return y


# revision 24
# speedup vs baseline: 1.1224x; 1.0549x over previous
"""LBLHighwayBiLm Trainium2 kernel (8-core data-parallel over batch).

v2: the 5-tap causal/anticausal convs run on the PE as banded token-mixing
matmuls. Activations for the conv live in token-partition layout ("x_T":
128 tokens on partitions, blocks x 256 chans on free dim); each 128-token
block is one [K=128] band matmul plus a [K=4] fixup matmul whose rhs is
either the neighbouring block's edge tokens or the (constant) pad vectors.
Conv PSUM is evicted to bf16 SBUF (DVE) and DMA-transposed back to
channel-partition layout for the highway matmuls.

Highway sublayers: W^T stationary bf16 matmuls into PSUM; sigmoid/relu
(+bias) evict PSUM on ACT; combine t=x-r, x1=u+r on DVE, u=g*t on GPSIMD
via the STT form (cheaper than TT there). fwd/bwd instruction emission is
interleaved to keep every engine (esp. PE pstate) busy.

Each core handles 4 batch rows; no collectives.
"""

import numpy as np
import ml_dtypes

import concourse.bacc as bacc
import concourse.tile as tile
import concourse.mybir as mybir
from concourse.bass_utils import run_bass_kernel_spmd

BF16 = mybir.dt.bfloat16
F32 = mybir.dt.float32
AOP = mybir.AluOpType
AFT = mybir.ActivationFunctionType

N_LAYERS = 2
N_HW = 2
W = 4
D = 256
B, S = 32, 1024
NCORES = 8
BLOC = B // NCORES          # 4 batch rows per core
T = BLOC * S                # 4096 tokens per core
PB = D // 128               # 2 partition blocks for D
EB = (2 * D) // 128         # 4 partition blocks for 2D
CH = 1024                   # token chunk = one batch row
NBLK = T // 128             # 32 token blocks of 128
BPR = S // 128              # 8 blocks per batch row

# --- engine assignment knobs -------------------------------------------------
ENG_T = ["v", "v", "v", "v"]       # t = x0 - r, per tg
ENG_U = ["g", "g", "g", "v"]       # u = g * t
ENG_X1 = ["v", "v", "v", "v"]      # x1 = u + r
RELU_ENGINE = ["a", "a", "a", "a"]  # per tg: "a"=ACT, "v"=DVE tensor_scalar
EVICT_ENGINE = "v"                  # conv psum evict


def _tt(nc, code, out, a, b, op):
    """tensor_tensor on DVE ("v") or gpsimd ("g")."""
    eng = nc.vector if code == "v" else nc.gpsimd
    eng.tensor_tensor(out, a, b, op)


def build_bass(params):
    """params: dict of host-precomputed constant arrays / floats."""
    nc = bacc.Bacc(target_bir_lowering=False)

    x_in = nc.dram_tensor("x", [128, NBLK * 256], BF16, kind="ExternalInput")
    out = nc.dram_tensor(
        "out", [N_LAYERS, 2, PB, 128, T], BF16, kind="ExternalOutput"
    )

    # inline constants
    wt_dram = nc.inline_tensor(params["wt"], name="wt")        # [L,2,HW,PB,128,512] bf16
    bias_dram = nc.inline_tensor(params["bias"], name="bias")  # [128, L*2*HW*EB] f32
    band_dram = nc.inline_tensor(params["band"], name="band")  # [L,2,2,128,128] bf16
    pad_dram = nc.inline_tensor(params["pad"], name="pad")     # [L,2,4,256] bf16

    with tile.TileContext(nc) as tc:
        consts = tc.alloc_tile_pool(name="consts", bufs=1)
        bufs = tc.alloc_tile_pool(name="bufs", bufs=1)
        scratch = tc.alloc_tile_pool(name="scratch", bufs=5)
        psum = tc.alloc_tile_pool(name="psum", bufs=4, space="PSUM")

        # ---- load constants (conv deps first so PE can start early) ---------
        xT0 = bufs.tile([128, NBLK * 256], BF16, tag="xT0", name="xT0")
        nc.gpsimd.dma_start(out=xT0, in_=x_in[:, :])
        band_sb = {}   # (l, di, fix) -> [128,128] tile
        for l in range(N_LAYERS):
            for di in range(2):
                for fx in range(2):
                    bt = consts.tile([128, 128], BF16, tag=f"bd{l}{di}{fx}",
                                     name=f"bd{l}{di}{fx}")
                    nc.sync.dma_start(out=bt, in_=band_dram[l, di, fx])
                    band_sb[(l, di, fx)] = bt
        pad_sb = {}    # (l, di) -> pad tile; fwd pads on partitions 124..127,
        #                bwd pads on partitions 0..3 (rest zero)
        for l in range(N_LAYERS):
            for di in range(2):
                pt = consts.tile([128, 256], BF16, tag=f"pd{l}{di}",
                                 name=f"pd{l}{di}")
                nc.sync.dma_start(out=pt, in_=pad_dram[l, di])
                pad_sb[(l, di)] = pt
        wt_sb = {}
        for l in range(N_LAYERS):
            for di in range(2):
                for h in range(N_HW):
                    for kb in range(PB):
                        wtt = consts.tile(
                            [128, 2 * D], BF16, tag=f"wt{l}{di}{h}{kb}",
                            name=f"wt{l}{di}{h}{kb}",
                        )
                        nc.scalar.dma_start(out=wtt, in_=wt_dram[l, di, h, kb])
                        wt_sb[(l, di, h, kb)] = wtt
        bias_sb = consts.tile([128, N_LAYERS * 2 * N_HW * EB], F32, name="bias_sb")
        nc.scalar.dma_start(out=bias_sb, in_=bias_dram[:, :])

        def bias_ap(l, di, h, eb):
            i = ((l * 2 + di) * N_HW + h) * EB + eb
            return bias_sb[:, i:i + 1]

        # ---- conv on PE -----------------------------------------------------
        def conv_group(l, di, xT, g, fT):
            """One psum group: token blocks [4g, 4g+4) of the banded conv.
            Evicts into fT halves [128, NBLK*128] at [:, g*512:(g+1)*512].
            fwd fixup reads the previous block's partitions 64..127 (only
            124..127 carry nonzero lhsT rows) so the rhs base partition is
            a legal 64; row-start blocks read the pad tile instead."""
            ps = psum.tile([128, 1024], F32, tag="ps", name=f"cps{l}{di}{g}")
            main = band_sb[(l, di, 0)]
            fix = band_sb[(l, di, 1)]
            for i in range(4):
                b = 4 * g + i
                nc.tensor.matmul(
                    ps[:, i * 256:(i + 1) * 256],
                    lhsT=main, rhs=xT[:, b * 256:(b + 1) * 256],
                    start=True, stop=False,
                )
                if di == 0:
                    if b % BPR == 0:
                        rhs = pad_sb[(l, 0)][64:128, :]
                    else:
                        rhs = xT[64:128, (b - 1) * 256:b * 256]
                    lhsT_f = fix[64:128, :]
                else:
                    if b % BPR == BPR - 1:
                        rhs = pad_sb[(l, 1)][:4, :]
                    else:
                        rhs = xT[0:4, (b + 1) * 256:(b + 2) * 256]
                    lhsT_f = fix[:4, :]
                nc.tensor.matmul(
                    ps[:, i * 256:(i + 1) * 256],
                    lhsT=lhsT_f, rhs=rhs,
                    start=False, stop=True,
                )
            # evict both chan-halves: fT_h[t, b*128+c] = ps[t, (b%4)*256+h*128+c]
            for h in range(2):
                src = ps.rearrange("p (i c) -> p i c", c=256)[:, :, h * 128:(h + 1) * 128]
                dst = fT[h][:, g * 512:(g + 1) * 512].rearrange(
                    "p (i c) -> p i c", c=128)
                if EVICT_ENGINE == "v":
                    nc.vector.tensor_copy(dst, src)
                else:
                    nc.scalar.copy(dst, src)

        # ---- one highway sublayer chunk ------------------------------------
        def hw_chunk(l, di, h, tg, x0, x1):
            """One 1024-token chunk of highway sublayer (l, di, h).
            Nonlin matmuls + relu evictions are emitted before the gate
            matmuls so PSUM slots recycle quickly (2-chunk pipelining)."""
            def mm(eb):
                p = psum.tile([128, 1024], F32, tag="ps", name=f"hps{l}{di}{h}{tg}{eb}")
                for half in range(2):
                    for kb in range(PB):
                        nc.tensor.matmul(
                            p[:, half * 512:(half + 1) * 512],
                            lhsT=wt_sb[(l, di, h, kb)][:, eb * 128:(eb + 1) * 128],
                            rhs=x0[kb][:, tg * CH + half * 512:tg * CH + (half + 1) * 512],
                            start=(kb == 0),
                            stop=(kb == PB - 1),
                        )
                return p

            gt, rt = {}, {}
            pss = {eb: mm(eb) for eb in (0, 1)}
            for blk in range(PB):
                rt[blk] = scratch.tile([128, CH], BF16, tag="r", name=f"r{l}{di}{h}{blk}{tg}")
                if RELU_ENGINE[tg] == "a":
                    nc.scalar.activation(
                        rt[blk], pss[blk], AFT.Relu,
                        bias=bias_ap(l, di, h, blk), scale=1.0,
                    )
                else:
                    nc.vector.tensor_scalar(
                        rt[blk], pss[blk], bias_ap(l, di, h, blk), 0.0,
                        AOP.add, AOP.max,
                    )
            for blk in range(PB):
                pg = mm(PB + blk)
                gt[blk] = scratch.tile([128, CH], BF16, tag="g", name=f"g{l}{di}{h}{blk}{tg}")
                nc.scalar.activation(
                    gt[blk], pg, AFT.Sigmoid,
                    bias=bias_ap(l, di, h, PB + blk), scale=1.0,
                )
            for blk in range(PB):
                x0c = x0[blk][:, tg * CH:(tg + 1) * CH]
                tt = scratch.tile([128, CH], BF16, tag="t", name=f"t{l}{di}{h}{blk}{tg}")
                _tt(nc, ENG_T[tg], tt, x0c, rt[blk], AOP.subtract)
                ut = scratch.tile([128, CH], BF16, tag="u", name=f"u{l}{di}{h}{blk}{tg}")
                _tt(nc, ENG_U[tg], ut, gt[blk], tt, AOP.mult)
                x1c = x1[blk][:, tg * CH:(tg + 1) * CH]
                _tt(nc, ENG_X1[tg], x1c, ut, rt[blk], AOP.add)

        # ---- the network ----------------------------------------------------
        xT = {0: xT0, 1: xT0}
        for l in range(N_LAYERS):
            # conv: banded matmuls, fwd/bwd interleaved
            fT = {}
            f = {}
            for di in range(2):
                fT[di] = [
                    bufs.tile([128, NBLK * 128], BF16, tag=f"fT{di}{h}",
                              name=f"fT{l}{di}{h}")
                    for h in range(2)
                ]
                f[di] = [
                    bufs.tile([128, T], BF16, tag=f"f{di}{blk}", name=f"f{l}{di}{blk}")
                    for blk in range(PB)
                ]
            for g in range(NBLK // 4):
                for di in range(2):
                    conv_group(l, di, xT[di], g, fT[di])
                if g % 2 == 1:
                    tg = g // 2
                    for di in range(2):
                        for h in range(2):
                            nc.sync.dma_start_transpose(
                                out=f[di][h][:, tg * CH:(tg + 1) * CH]
                                    .rearrange("p (a b) -> p a b", b=128),
                                in_=fT[di][h][:, tg * CH:(tg + 1) * CH],
                            )

            # highway sublayer A
            xa = {}
            for di in range(2):
                xa[di] = [
                    bufs.tile([128, T], BF16, tag=f"xa{di}{blk}", name=f"xa{l}{di}{blk}")
                    for blk in range(PB)
                ]
            for tg in range(4):
                for di in range(2):
                    hw_chunk(l, di, 0, tg, f[di], xa[di])

            # highway sublayer B (+ output store, + next-layer transposes)
            xb = {}
            # xb reuses the f buffers (dead after sublayer A)
            for di in range(2):
                xb[di] = [
                    bufs.tile([128, T], BF16, tag=f"f{di}{blk}", name=f"xb{l}{di}{blk}")
                    for blk in range(PB)
                ]
                if l + 1 < N_LAYERS:
                    xT[di] = bufs.tile([128, NBLK * 256], BF16,
                                       tag="xT0" if di == 0 else "xT1b",
                                       name=f"xT1{di}")
            for tg in range(4):
                for di in range(2):
                    hw_chunk(l, di, 1, tg, xa[di], xb[di])
                    if l + 1 < N_LAYERS:
                        for h in range(2):
                            nc.sync.dma_start_transpose(
                                out=xT[di].rearrange("p (a b) -> p a b", b=256)
                                    [:, tg * BPR:(tg + 1) * BPR, h * 128:(h + 1) * 128],
                                in_=xb[di][h][:, tg * CH:(tg + 1) * CH],
                            )

                    for blk in range(PB):
                        nc.sync.dma_start(
                            out=out[l, di, blk].rearrange("p (r c) -> p r c", c=CH)[:, tg],
                            in_=xb[di][blk][:, tg * CH:(tg + 1) * CH],
                        )

        psum.release()
        scratch.release()
        bufs.release()
        consts.release()

    nc.finalize()
    return nc


def _prep_params(inputs):
    fwd_hw_W = np.asarray(inputs["fwd_hw_W"], np.float32)
    bwd_hw_W = np.asarray(inputs["bwd_hw_W"], np.float32)
    # lhsT layout: [l, dir, hw, kb, 128(k), 2D(e)] = W[e, k] transposed
    wt = np.empty((N_LAYERS, 2, N_HW, PB, 128, 2 * D), np.float32)
    for l in range(N_LAYERS):
        for di, Wsrc in ((0, fwd_hw_W), (1, bwd_hw_W)):
            for h in range(N_HW):
                wT = Wsrc[l, h].T  # [D, 2D]
                wt[l, di, h] = wT.reshape(PB, 128, 2 * D)
    wt = wt.astype(ml_dtypes.bfloat16)

    fwd_hw_b = np.asarray(inputs["fwd_hw_b"], np.float32)
    bwd_hw_b = np.asarray(inputs["bwd_hw_b"], np.float32)
    bias = np.empty((128, N_LAYERS * 2 * N_HW * EB), np.float32)
    for l in range(N_LAYERS):
        for di, bsrc in ((0, fwd_hw_b), (1, bwd_hw_b)):
            for h in range(N_HW):
                for eb in range(EB):
                    i = ((l * 2 + di) * N_HW + h) * EB + eb
                    bias[:, i] = bsrc[l, h, eb * 128:(eb + 1) * 128]

    # banded conv weights: main [128,128] + fixup [4,128] (stored in 128 rows)
    fwd_w = np.asarray(inputs["fwd_w"], np.float32)
    bwd_w = np.asarray(inputs["bwd_w"], np.float32)
    band = np.zeros((N_LAYERS, 2, 2, 128, 128), np.float32)
    for l in range(N_LAYERS):
        for e in range(128):
            for p in range(128):
                d = p - e
                if -W <= d <= 0:
                    band[l, 0, 0, p, e] = fwd_w[l, d + W]   # w_{p-e+4}
                if 0 <= d <= W:
                    band[l, 1, 0, p, e] = bwd_w[l, d]       # v_{p-e}
        for e in range(128):
            for k in range(4):
                j = k - e
                if 0 <= j <= 3:
                    # fwd fixup rows live at partitions 124..127 (rhs base 64)
                    band[l, 0, 1, 124 + k, e] = fwd_w[l, j]  # w_{k-e}
                j = 128 + k - e
                if 1 <= j <= 4:
                    band[l, 1, 1, k, e] = bwd_w[l, j]        # v_{128+k-e}
    band = band.astype(ml_dtypes.bfloat16)

    fwd_pad = np.asarray(inputs["fwd_pad"], np.float32)
    bwd_pad = np.asarray(inputs["bwd_pad"], np.float32)
    pad = np.zeros((N_LAYERS, 2, 128, 256), np.float32)
    pad[:, 0, 124:128] = fwd_pad      # fwd pads under the 124..127 lhsT rows
    pad[:, 1, 0:4] = bwd_pad
    pad = np.ascontiguousarray(pad).astype(ml_dtypes.bfloat16)

    return {
        "wt": np.ascontiguousarray(wt),
        "bias": np.ascontiguousarray(bias),
        "band": np.ascontiguousarray(band),
        "pad": pad,
    }


def _prep_core_input(x_core):
    """x_core: [BLOC, S, D] f32 -> x_T [128, NBLK*256] bf16 token layout."""
    # x_T[t, b*256 + c] = x_core[b // BPR, (b % BPR)*128 + t, c]
    xt = x_core.reshape(BLOC * BPR, 128, D)          # [b, t, c]
    xt = np.ascontiguousarray(xt.transpose(1, 0, 2))  # [t, b, c]
    return xt.reshape(128, NBLK * D).astype(ml_dtypes.bfloat16)


_NC_CACHE = {}


def kernel(**inputs):
    params = _prep_params(inputs)
    import hashlib
    h = hashlib.sha256()
    for k in ("wt", "bias", "band", "pad"):
        h.update(np.ascontiguousarray(params[k]).tobytes())
    key = h.hexdigest()
    if key not in _NC_CACHE:
        _NC_CACHE[key] = build_bass(params)
    nc = _NC_CACHE[key]

    x = np.asarray(inputs["inputs"], np.float32)
    in_maps = [
        {"x": _prep_core_input(x[c * BLOC:(c + 1) * BLOC])}
        for c in range(NCORES)
    ]
    res = run_bass_kernel_spmd(nc, in_maps, core_ids=list(range(NCORES)))

    y = np.empty((N_LAYERS, B, S, 2 * D), np.float32)
    for c in range(NCORES):
        o = np.asarray(res.results[c]["out"]).astype(np.float32)
        # [L, dir, blk, p, T] -> [L, r, s, dir*256+blk*128+p]
        o = o.reshape(N_LAYERS, 2, PB, 128, BLOC, S)
        o = o.transpose(0, 4, 5, 1, 2, 3).reshape(N_LAYERS, BLOC, S, 2 * D)
        y[:, c * BLOC:(c + 1) * BLOC] = o
    return y
